# revision 61
# baseline (speedup 1.0000x reference)
"""Trainium2 Bass kernel for nn_CDAN_Dis (CDAN discriminator head), v5.

Math per sample m (see reference):
  a    = einsum('cf,bft->bct', w2d, feature)            # [C,T]
  d    = einsum('bct,bcpt->bpt', a, mask) + b2d         # [P,T]
  d    = leaky(GLN_scalar(d))                           # global LN over (P,T)
  x1   = leaky(GLN_vec(conv1d(d,  w1,b1, s2,p1)))       # [256,1000]
  x2   = leaky(GLN_vec(conv1d(x1, w2,b2, s2,p1)))       # [256,500]
  out  = conv1d(x2, w3, b3, s1, p0)                     # [1,500]

Data-parallel over batch M=4 across 4 NeuronCores (one sample per core).

v6 design (HW-validated):
 - Stage-1 stays channel-major: a[c,:] = sum_f w2d[c,f]*feat[f,:] is
   computed BROADCAST across partitions by a gpsimd partition_all_reduce
   of the w2d-weighted feature (two 4x tensor_scalar passes feed two
   [128,2000] all-reduces on the otherwise idle Pool engine).  The mask
   multiplies and add are then plain 2x f16 tensor_tensor ops; GLN1
   sum/sumsq ride free accum_outs of 4x tensor_scalar passes.  No PE, no
   PSUM, no transposes anywhere in stage-1.
 - DVE perf-mode economics (cost model): tensor_scalar packed f16 SBUF
   runs 4x, tensor_tensor 2x, scalar_tensor_tensor/tensor_reduce always
   1x -- op selection above follows from this.
 - GLN chains: tiny DVE column folds + one gpsimd partition_all_reduce
   (no PE matmul, no PSUM slot, so the three chains never serialize),
   then 4 ACT ops + DVE recip for mean/var/rstd/scale/bias.
 - conv1 writes per-oh 2-bank [128,1024] psum tiles (bank-aligned 512/488
   matmul halves, contiguous 0:1000), so norm2 is ONE wide ACT Prelu per
   oh; sumsq Squares run per half right behind their matmuls.  conv2
   packs both oh halves into ONE 2-bank tile at offsets 0/512 (merged
   per-half sum reduce); conv3 reuses that tile's dead first 500 cols.
 - conv1 sums use the S_even/S_odd trick (strided 2x tensor_scalar accum
   passes over xpad, dotted with host col-summed w1).
 - Constant weights (cw/cwf) are DMA'd once in a prologue and stay
   resident; per-iteration input DMAs are issued from the Pool queue.
 - No bias matmuls: conv1/conv2 biases ride the ACT Square bias operand
   (sumsq of y+b) and fold into the GLN normalize bias; conv3's b3 rides
   the PSUM->SBUF output copy.  Bias SUM contributions are host
   constants (cb1/cb2).
 - 2-deep software pipeline; the tile scheduler is dependency-driven
   (emission order only sets priority), so the structure above mainly
   breaks cross-iteration tile-slot recurrences: psc1 bufs=3 decouples
   conv1(i+1) from norm2(i), xpadp bufs=3 decouples norm1 from old conv1
   readers.

Engine notes (the real compiler rejects what CoreSim happily simulates):
 - gpsimd/Pool only runs plain tensor add/mul/copy + partition_all_reduce
   (no accum_out, no tensor_scalar, no PSUM operands); DVE
   tensor_tensor_reduce crashes the device; DVE may read at most one
   PSUM operand.
 - b2d is a uniform additive constant immediately followed by a global
   layernorm, so it cancels exactly and is ignored.
"""

import sys

sys.path.insert(0, "/opt/trn_rl_repo")

from contextlib import ExitStack

import numpy as np

import concourse.bass as bass
import concourse.bass_isa as bass_isa
import concourse.mybir as mybir
import concourse.tile as tile
from concourse import bacc, bass_utils

F32 = mybir.dt.float32
F16 = mybir.dt.float16
AX = mybir.AxisListType
OP = mybir.AluOpType
AF = mybir.ActivationFunctionType

M, C, B, T = 4, 2, 128, 2000
TC = 500               # matmul free-dim chunk (PSUM bank limit)
T1 = 1000              # conv1 output length
T2 = 500               # conv2 output length
EPS = 1e-8

NB = 16                # time blocks in transposed layout
TB = NB * 128          # 2048 (T=2000 zero-padded)
NVB = 15               # fully-valid blocks (cols 0:1920)

N1 = B * T             # GLN1 element count
N2 = 256 * T1
N3 = 256 * T2

USE_PRELU = True       # fused affine+leaky on ACT (Prelu alpha=0.1)
N_CORES = 4

# packed fp16 weights column offsets (CW: [128, CWW] f16)
CW_W2DT = 0            # w2d^T [128, 2]
CW_W1T = 2             # w1 transposed [128, 768]
CW_W2T = 770           # w2 transposed [128, 1536]
CW_W3T = 2306          # w3 cols + pad for 128-col lhsT reads
CW_W2DR = 2438         # w2d broadcast lhsT [128, 256]
CWW = 2694
# packed f32 per-partition constants (CWF: [128, CWF_W] f32)
CF_ONES = 0            # ones [128, 128] (stats-bcast lhsT)
CF_W1S = 128           # w1.sum(out_ch) [128, 3]
CF_G1 = 131            # gamma1 per oh-half [128, 2]
CF_BB1 = 133
CF_G2 = 135
CF_BB2 = 137
CF_EPS = 139
CF_CB1 = 140           # T1*sum(b1)/128
CF_CB2 = 141           # T2*sum(b2)/128
CF_G2D = 142
CF_BE2D = 143
CF_B3 = 144
CF_B1C = 145           # b1 per oh-half [128, 2]
CF_B2C = 147           # b2 per oh-half [128, 2]
CF_W2D = 149           # w2d rows as per-partition scalars [128, 2]
CWF_W = 151

# packed input row (f16): feat | m0 | m1 (all channel-major [128, 2000])
IN_F = 0
IN_M0 = T
IN_M1 = 2 * T
INW = 3 * T            # 6000


def _patch_act_tables():
    """Pin every ACT func we use to the one set that has them all."""
    if getattr(bacc, "_cdan_act_patch", False):
        return
    orig = bacc.get_activation_tables
    mine = {AF.Copy, AF.Identity, AF.Square, AF.Sqrt, AF.Prelu}

    def patched(arch):
        t = dict(orig(arch))
        for name in t:
            if name != "sqrt_and_others":
                t[name] = set(t[name]) - mine
        return t

    bacc.get_activation_tables = patched
    bacc._cdan_act_patch = True


def build_nc(repeat=1):
    _patch_act_tables()
    nc = bacc.Bacc("TRN2", target_bir_lowering=False, debug=False,
                   num_devices=N_CORES)

    inp_d = nc.dram_tensor("inp", [128, INW], F16, kind="ExternalInput").ap()
    cw_d = nc.dram_tensor("cw", [128, CWW], F16, kind="ExternalInput").ap()
    cwf_d = nc.dram_tensor("cwf", [128, CWF_W], F32, kind="ExternalInput").ap()
    out_d = nc.dram_tensor("out", [1, T2], F32, kind="ExternalOutput").ap()

    with tile.TileContext(nc) as tc:
        with ExitStack() as ctx:
            pools = _make_pools(ctx, tc)
            cst = _emit_prologue(pools, tc, cw_d, cwf_d)
            prev = None
            for _ in range(repeat):
                st = _emit_A_dmas(pools, tc, cst, inp_d, out_d)
                if prev is None:
                    _emit_A_stage1(pools, tc, st)
                    _emit_chain1(pools, tc, st)
                    _emit_norm1(pools, tc, st)
                else:
                    gen = _emit_B_tail(pools, tc, prev)
                    next(gen)              # conv1 + V1 of prev
                    next(gen)              # chain2 of prev
                    next(gen)              # norm2 of prev
                    next(gen)              # conv2 + stats of prev
                    next(gen)              # chain3 of prev
                    _emit_A_stage1(pools, tc, st)
                    _emit_chain1(pools, tc, st)
                    next(gen)              # norm3 of prev
                    _emit_norm1(pools, tc, st)
                    for _ in gen:          # conv3 + out of prev
                        pass
                prev = st
            for _ in _emit_B_tail(pools, tc, prev):
                pass
    nc.compile()
    return nc


def _make_pools(ctx, tc):
    class P:
        pass
    p = P()
    p.const = ctx.enter_context(tc.tile_pool(name="const", bufs=1))
    p.inp = ctx.enter_context(tc.tile_pool(name="inp", bufs=2))
    p.tmpT = ctx.enter_context(tc.tile_pool(name="tmpT", bufs=2))
    p.tmpp = ctx.enter_context(tc.tile_pool(name="tmpp", bufs=4))
    p.sqp = ctx.enter_context(tc.tile_pool(name="sqp", bufs=2))
    p.bigp = ctx.enter_context(tc.tile_pool(name="bigp", bufs=2))
    p.smallp = ctx.enter_context(tc.tile_pool(name="smallp", bufs=2))
    p.xpadp = ctx.enter_context(tc.tile_pool(name="xpadp", bufs=3))
    p.psc1 = ctx.enter_context(tc.tile_pool(name="psc1", bufs=2, space="PSUM"))
    p.psc2 = ctx.enter_context(tc.tile_pool(name="psc2", bufs=1, space="PSUM"))
    p.psa = ctx.enter_context(tc.tile_pool(name="psa", bufs=2, space="PSUM"))
    return p


def _gln_chain(nc, pools, stab, onesq, epsc, inv_n, tag):
    """stab [128,2w] = (sums 0:w | sumsqs w:2w) -> (rstd, negmean).

    Column folds on DVE (tiny ts accums), then ONE gpsimd
    partition_all_reduce gives the totals on every partition -- no PE
    matmul, no PSUM, so the three chains never serialize on a psum slot.
    The mean/var/sstd math then runs as 4 consecutive ACT ops."""
    smallp = pools.smallp
    w = stab[:].shape[-1] // 2
    if w == 1:
        sq2 = stab[:]
    else:
        sq2t = smallp.tile([128, 2], F32, tag=f"f{tag}")
        dsc = smallp.tile([128, w], F32, tag=f"fd{tag}")
        nc.vector.tensor_scalar(dsc[:], stab[:, 0:w], 1.0, 0.0,
                                OP.mult, OP.add, accum_out=sq2t[:, 0:1])
        nc.vector.tensor_scalar(dsc[:], stab[:, w:2 * w], 1.0, 0.0,
                                OP.mult, OP.add, accum_out=sq2t[:, 1:2])
        sq2 = sq2t[:]
    red = smallp.tile([128, 2], F32, tag=f"red{tag}")
    nc.gpsimd.partition_all_reduce(red[:], sq2, 128, bass_isa.ReduceOp.add)
    mE = smallp.tile([128, 2], F32, tag=f"mE{tag}")
    nc.scalar.activation(mE[:], red[:], AF.Copy, bias=0.0, scale=-inv_n)
    sqm = smallp.tile([128, 1], F32, tag=f"sqm{tag}")
    nc.scalar.activation(sqm[:], mE[:, 0:1], AF.Square)
    nvar = smallp.tile([128, 1], F32, tag=f"nvar{tag}")
    nc.scalar.activation(nvar[:], sqm[:], AF.Identity,
                         bias=mE[:, 1:2], scale=1.0)      # mean^2 - E2
    sstd = smallp.tile([128, 1], F32, tag=f"sstd{tag}")
    nc.scalar.activation(sstd[:], nvar[:], AF.Sqrt, bias=epsc, scale=-1.0)
    rstd = smallp.tile([128, 1], F32, tag=f"rstd{tag}")
    nc.vector.reciprocal(rstd[:], sstd[:])
    return rstd, mE[:, 0:1]


def _scale_bias(nc, pools, rstd, negmean, gam, bet, w, tag, bcol=None):
    """scale = gamma*rstd; bias = scale*(bcol - mean) + beta   ([128, w])."""
    smallp = pools.smallp
    sc = smallp.tile([128, w], F32, tag=f"sc{tag}")
    nc.vector.tensor_scalar_mul(sc[:], gam, rstd)
    bi = smallp.tile([128, w], F32, tag=f"bi{tag}")
    if bcol is None:
        nc.vector.scalar_tensor_tensor(bi[:], sc[:], negmean, bet,
                                       OP.mult, OP.add)
    else:
        nmb = smallp.tile([128, w], F32, tag=f"nmb{tag}")
        nc.vector.tensor_scalar_add(nmb[:], bcol, negmean)
        sb = smallp.tile([128, w], F32, tag=f"sb{tag}")
        nc.vector.tensor_mul(sb[:], sc[:], nmb[:])
        nc.vector.tensor_add(bi[:], sb[:], bet)
    return sc, bi


def _norm_leaky(nc, pools, out_ap, in_ap, scale_ap, bias_ap, accum=None):
    """out = leaky(in*scale + bias), slope 0.1; optional sum accumulator."""
    if USE_PRELU:
        nc.scalar.activation(out_ap, in_ap, AF.Prelu,
                             bias=bias_ap, scale=scale_ap, alpha=0.1,
                             accum_out=accum)
    else:
        af = pools.tmpp.tile([128, in_ap.shape[-1]], F32, tag="nl")
        nc.scalar.activation(af[:], in_ap, AF.Identity,
                             bias=bias_ap, scale=scale_ap)
        nc.vector.scalar_tensor_tensor(out_ap, af[:], 0.1, af[:],
                                       OP.mult, OP.max, accum_out=accum)


SECTION_LOG = None


def _mark(nc, label):
    if SECTION_LOG is not None:
        SECTION_LOG.append((label, nc.next_id()))


class _St:
    """Per-iteration emission state."""


def _emit_prologue(pools, tc, cw_d, cwf_d):
    """One-time constant loads; tiles stay resident across iterations."""
    nc = tc.nc
    cst = _St()
    cw = pools.const.tile([128, CWW], F16, tag="cw")
    nc.sync.dma_start(cw[:], cw_d[:])
    cwf = pools.const.tile([128, CWF_W], F32, tag="cwf")
    nc.sync.dma_start(cwf[:], cwf_d[:])
    cst.cw, cst.cwf = cw, cwf
    cst.w2dc = cwf[:, CF_W2D:CF_W2D + 2]
    cst.w2dr = cw[:, CW_W2DR:CW_W2DR + 256]
    cst.w1t = cw[:, CW_W1T:CW_W1T + 768]
    cst.w2t = cw[:, CW_W2T:CW_W2T + 1536]
    cst.onesq = cwf[:, CF_ONES:CF_ONES + 128]
    cst.w1s = cwf[:, CF_W1S:CF_W1S + 3]
    cst.g1f = cwf[:, CF_G1:CF_G1 + 2]
    cst.bb1f = cwf[:, CF_BB1:CF_BB1 + 2]
    cst.g2f = cwf[:, CF_G2:CF_G2 + 2]
    cst.bb2f = cwf[:, CF_BB2:CF_BB2 + 2]
    cst.epsc = cwf[:, CF_EPS:CF_EPS + 1]
    cst.cb1c = cwf[:, CF_CB1:CF_CB1 + 1]
    cst.cb2c = cwf[:, CF_CB2:CF_CB2 + 1]
    cst.g2dc = cwf[:, CF_G2D:CF_G2D + 1]
    cst.be2dc = cwf[:, CF_BE2D:CF_BE2D + 1]
    cst.b3c = cwf[:, CF_B3:CF_B3 + 1]
    cst.b1c = cwf[:, CF_B1C:CF_B1C + 2]
    cst.b2c = cwf[:, CF_B2C:CF_B2C + 2]
    return cst


def _emit_A_dmas(pools, tc, cst, inp_d, out_d):
    """Emit iteration i's input DMA triggers (on the Pool queue)."""
    nc = tc.nc
    st = _St()
    st.__dict__.update(cst.__dict__)
    _mark(nc, "A_dmas")
    st.out_d = out_d
    inp = pools.inp.tile([128, INW], F16, tag="inp")
    nc.gpsimd.dma_start(inp[:, IN_F:IN_F + T], inp_d[:, IN_F:IN_F + T])
    nc.gpsimd.dma_start(inp[:, IN_M0:IN_M0 + T], inp_d[:, IN_M0:IN_M0 + T])
    nc.gpsimd.dma_start(inp[:, IN_M1:IN_M1 + T], inp_d[:, IN_M1:IN_M1 + T])
    st.inp = inp
    st.feat = inp[:, IN_F:IN_F + T]
    st.m0 = inp[:, IN_M0:IN_M0 + T]
    st.m1 = inp[:, IN_M1:IN_M1 + T]
    st.stab1 = pools.smallp.tile([128, 2], F32, tag="stab1")
    return st


def _b3d(ap, q=128):
    return ap.rearrange("p (b q) -> p b q", q=q)


def _emit_A_stage1(pools, tc, st):
    """Stage-1, channel-major: a[c,:] broadcast across partitions by PE
    ones-style matmuls (lhsT = w2d[c] replicated into 128 columns), mask
    multiplies read the PSUM broadcast directly; full-width 2x add /
    square and 4x tensor_scalar accum passes for the GLN1 stats."""
    nc = tc.nc
    _mark(nc, "A_s1")
    t0 = pools.tmpT.tile([128, T], F16, tag="t0")
    t1 = pools.tmpT.tile([128, T], F16, tag="t1")
    for j in range(4):
        sl = slice(j * TC, (j + 1) * TC)
        a0 = pools.psa.tile([128, TC], F32, tag="aps")
        nc.tensor.matmul(a0[:], st.w2dr[:, 0:128], st.feat[:, sl],
                         start=True, stop=True)
        a1 = pools.psa.tile([128, TC], F32, tag="aps")
        nc.tensor.matmul(a1[:], st.w2dr[:, 128:256], st.feat[:, sl],
                         start=True, stop=True)
        nc.vector.tensor_tensor(t0[:, sl], st.m0[:, sl], a0[:], OP.mult)
        nc.vector.tensor_tensor(t1[:, sl], st.m1[:, sl], a1[:], OP.mult)
    dT = pools.bigp.tile([128, T], F16, tag="dT")
    nc.vector.tensor_tensor(dT[:], t0[:], t1[:], OP.add)
    sdisc = pools.sqp.tile([128, T], F16, tag="zq")
    nc.vector.tensor_scalar(sdisc[:], dT[:], 1.0, 0.0, OP.mult, OP.add,
                            accum_out=st.stab1[:, 0:1])
    sqT = pools.sqp.tile([128, T], F16, tag="sq")
    nc.vector.tensor_tensor(sqT[:], dT[:], dT[:], OP.mult)
    nc.vector.tensor_scalar(sdisc[:], sqT[:], 1.0, 0.0, OP.mult, OP.add,
                            accum_out=st.stab1[:, 1:2])
    st.dT = dT


def _emit_chain1(pools, tc, st):
    """GLN1 stats chain for iteration st (emitted during prev's tail)."""
    nc = tc.nc
    _mark(nc, "chain1")
    rstd1, nm1 = _gln_chain(nc, pools, st.stab1, st.onesq, st.epsc,
                            1.0 / N1, "1")
    st.sc1, st.bi1 = _scale_bias(nc, pools, rstd1, nm1, st.g2dc, st.be2dc,
                                 1, "1")


def _emit_norm1(pools, tc, st):
    """GLN1 normalize+leaky straight into xpad, split in halves so conv1's
    first-half matmuls can start while the second half still computes."""
    nc = tc.nc
    sc1, bi1 = st.sc1, st.bi1
    _mark(nc, "norm1")
    xpad = pools.xpadp.tile([128, 2 + T], F16, tag="xpad")
    nc.vector.memset(xpad[:, 0:1], 0.0)
    nc.vector.memset(xpad[:, 1 + T:2 + T], 0.0)
    H = 1024
    for h, (lo, n) in enumerate(((0, H), (H, T - H))):
        _norm_leaky(nc, pools, xpad[:, 1 + lo:1 + lo + n],
                    st.dT[:, lo:lo + n], sc1[:, 0:1], bi1[:, 0:1])
    st.xpad = xpad


def _emit_B_tail(pools, tc, st):
    """Generator: conv1 through conv3/out for iteration st, yielding at the
    interleave points where the next iteration's stage-1 and chain1/norm1
    slot in."""
    nc = tc.nc
    sqp, bigp, smallp = pools.sqp, pools.bigp, pools.smallp
    xpad = st.xpad

    # ---- conv1 (128->256, k3 s2 p1); sumsq (with b1 bias) on ACT ----
    # each oh gets ONE 2-bank [128,1000] psum tile (matmuls fill 500-col
    # halves), so the Square and the norm2 normalize are single wide ops
    _mark(nc, "conv1")
    stab2 = smallp.tile([128, 8], F32, tag="stab2")
    py1 = {}
    # bank-aligned halves: t' 0..511 at psum offset 0, t' 512..999 at 512
    H1 = 512
    for tcb in range(2):
        lo = 0 if tcb == 0 else H1
        n = H1 if tcb == 0 else T1 - H1
        for oh in range(2):
            if tcb == 0:
                c1out = pools.psc1.tile([128, 1024], F32, tag="c1out")
                py1[oh] = c1out
            p = py1[oh]
            for k in range(3):
                rhs = xpad[:, k + 2 * lo: k + 2 * lo + 2 * n - 1:2]
                nc.tensor.matmul(p[:, lo:lo + n],
                                 st.w1t[:, k * 256 + oh * 128:
                                        k * 256 + oh * 128 + 128],
                                 rhs, start=(k == 0), stop=(k == 2))
            sq = sqp.tile([128, H1], F32, tag="sqa")
            nc.scalar.activation(sq[:, 0:n], p[:, lo:lo + n], AF.Square,
                                 bias=st.b1c[:, oh:oh + 1],
                                 accum_out=stab2[:, 4 + 2 * oh + tcb:
                                                 5 + 2 * oh + tcb])

    # conv1 sum-part: per-channel parity sums of x via 2 strided reduces,
    # dot with col-summed w1 (S = (SxO - xlast, SxE, SxO)).
    _mark(nc, "V1")
    # strided tensor_scalar accum passes (2x) beat tensor_reduce (1x)
    sx = smallp.tile([128, 2], F32, tag="sx")        # (SxE, SxO)
    vd = pools.tmpT.tile([128, T1], F16, tag="vd")
    nc.vector.tensor_scalar(vd[:], xpad[:, 1:2001:2], 1.0, 0.0, OP.mult,
                            OP.add, accum_out=sx[:, 0:1])
    nc.vector.tensor_scalar(vd[:], xpad[:, 2:2002:2], 1.0, 0.0, OP.mult,
                            OP.add, accum_out=sx[:, 1:2])
    s1t = smallp.tile([128, 3], F32, tag="s1t")
    nc.vector.tensor_sub(s1t[:, 0:1], sx[:, 1:2], xpad[:, 2000:2001])
    nc.vector.tensor_copy(s1t[:, 1:3], sx[:, 0:2])
    nc.vector.tensor_mul(stab2[:, 0:3], st.w1s[:], s1t[:])
    nc.vector.tensor_copy(stab2[:, 3:4], st.cb1c)
    yield

    # ---- GLN2 chain + normalize (b1 folded into the bias) ----
    _mark(nc, "chain2")
    rstd2, nm2 = _gln_chain(nc, pools, stab2, st.onesq, st.epsc,
                            1.0 / N2, "2")
    sc2, bi2 = _scale_bias(nc, pools, rstd2, nm2, st.g1f, st.bb1f, 2, "2",
                           bcol=st.b1c)
    yield

    # norm2: one wide ACT Prelu per oh
    _mark(nc, "norm2")
    y1pad = []
    for oh in range(2):
        yp = bigp.tile([128, T1 + 2], F16, tag=f"y1pad{oh}")
        y1pad.append(yp)
        nc.vector.memset(yp[:, 0:1], 0.0)
        _norm_leaky(nc, pools, yp[:, 1:1 + T1], py1[oh][:, 0:T1],
                    sc2[:, oh:oh + 1], bi2[:, oh:oh + 1])
    yield

    # ---- conv2 (256->256, k3 s2 p1); both oh halves in ONE 2-bank psum --
    _mark(nc, "conv2")
    stab3 = smallp.tile([128, 6], F32, tag="stab3")
    c2out = pools.psc2.tile([128, 1024], F32, tag="c2out")
    py2 = c2out
    for oh in range(2):
        lo = oh * 512                    # bank-aligned oh halves
        for cih in range(2):
            for k in range(3):
                rhs = y1pad[cih][:, k: k + 2 * T2 - 1:2]
                nc.tensor.matmul(py2[:, lo:lo + T2],
                                 st.w2t[:, cih * 768 + k * 256 + oh * 128:
                                        cih * 768 + k * 256 + oh * 128 + 128],
                                 rhs, start=(cih == 0 and k == 0),
                                 stop=(cih == 1 and k == 2))
        sqa = sqp.tile([128, TC], F32, tag="sqa2")
        nc.scalar.activation(sqa[:], py2[:, lo:lo + T2], AF.Square,
                             bias=st.b2c[:, oh:oh + 1],
                             accum_out=stab3[:, 3 + oh:4 + oh])
    # merged per-half sums; stab3 w=3: (s0, s1, cb2 | q0, q1, 0)
    nc.vector.reduce_sum(
        stab3[:, 0:2],
        py2[:].rearrange("p (b q) -> p b q", q=512)[:, :, 0:T2], axis=AX.X)
    nc.vector.tensor_copy(stab3[:, 2:3], st.cb2c)
    nc.vector.memset(stab3[:, 5:6], 0.0)
    yield

    # ---- GLN3 chain + normalize (b2 folded into the bias) ----
    _mark(nc, "chain3")
    rstd3, nm3 = _gln_chain(nc, pools, stab3, st.onesq, st.epsc,
                            1.0 / N3, "3")
    sc3, bi3 = _scale_bias(nc, pools, rstd3, nm3, st.g2f, st.bb2f, 2, "3",
                           bcol=st.b2c)
    yield

    _mark(nc, "norm3")
    x3 = []
    for oh in range(2):
        xt = bigp.tile([128, T2], F16, tag=f"x3_{oh}")
        x3.append(xt)
        _norm_leaky(nc, pools, xt[:], py2[:, oh * 512:oh * 512 + T2],
                    sc3[:, oh:oh + 1], bi3[:, oh:oh + 1])
    yield

    # ---- conv3 (256->1, k1) + b3 ----
    # lhsT is 128 consecutive CW columns whose col0 holds w3 for the half;
    # rows 1..127 of the psum accumulate garbage that we never read.  b3
    # rides the PSUM->SBUF output copy.
    # conv3 reuses the (now fully read) first 500 cols of the conv2 psum
    _mark(nc, "conv3")
    p3 = py2[:, 0:T2]
    nc.tensor.matmul(p3, st.cw[:, CW_W3T:CW_W3T + 128], x3[0][:],
                     start=True, stop=False)
    nc.tensor.matmul(p3, st.cw[:, CW_W3T + 1:CW_W3T + 129], x3[1][:],
                     start=False, stop=True)
    out_s = smallp.tile([1, T2], F32, tag="out_s")
    nc.scalar.activation(out_s[:], py2[0:1, 0:T2], AF.Identity,
                         bias=st.b3c[0:1, :], scale=1.0)
    nc.sync.dma_start(st.out_d[:], out_s[:])


def shard_inputs(inputs):
    """Full inputs -> per-core in_maps (host-side layout prep)."""
    f = {k: np.ascontiguousarray(np.asarray(v, dtype=np.float32))
         for k, v in inputs.items()}
    cw = np.zeros((128, CWW), np.float16)
    w2d = f["w2d"]
    cw[:, CW_W2DR:CW_W2DR + 128] = np.tile(w2d[0][:, None], (1, 128))
    cw[:, CW_W2DR + 128:CW_W2DR + 256] = np.tile(w2d[1][:, None], (1, 128))
    cw[:, CW_W1T:CW_W1T + 768] = f["w1"].transpose(1, 2, 0).reshape(128, 768)
    cw[:, CW_W2T:CW_W2T + 1536] = (
        f["w2"].transpose(1, 2, 0).reshape(2, 128, 3, 256)
        .transpose(1, 0, 2, 3).reshape(128, 1536))
    cw[:, CW_W3T:CW_W3T + 2] = f["w3"].reshape(2, 128).T

    cwf = np.zeros((128, CWF_W), np.float32)
    cwf[:, CF_ONES:CF_ONES + 128] = 1.0
    cwf[:, CF_W1S:CF_W1S + 3] = f["w1"].sum(axis=0)                # [128,3]
    cwf[:, CF_G1:CF_G1 + 2] = f["g1"].reshape(2, 128).T
    cwf[:, CF_BB1:CF_BB1 + 2] = f["bb1"].reshape(2, 128).T
    cwf[:, CF_G2:CF_G2 + 2] = f["g2"].reshape(2, 128).T
    cwf[:, CF_BB2:CF_BB2 + 2] = f["bb2"].reshape(2, 128).T
    cwf[:, CF_EPS] = EPS
    cwf[:, CF_CB1] = T1 * float(f["b1"].sum()) / 128.0
    cwf[:, CF_CB2] = T2 * float(f["b2"].sum()) / 128.0
    cwf[:, CF_G2D] = float(f["g2d"].reshape(()))
    cwf[:, CF_BE2D] = float(f["be2d"].reshape(()))
    cwf[:, CF_B3] = float(f["b3"].reshape(()))
    cwf[:, CF_B1C:CF_B1C + 2] = f["b1"].reshape(2, 128).T
    cwf[:, CF_B2C:CF_B2C + 2] = f["b2"].reshape(2, 128).T
    cwf[:, CF_W2D:CF_W2D + 2] = f["w2d"].T                 # [128f, 2c]

    in_maps = []
    for i in range(M):
        inp = np.empty((128, INW), np.float16)
        inp[:, IN_F:IN_F + T] = f["feature"][i].astype(np.float16)
        inp[:, IN_M0:IN_M0 + T] = f["mask"][i, 0].astype(np.float16)
        inp[:, IN_M1:IN_M1 + T] = f["mask"][i, 1].astype(np.float16)
        in_maps.append(dict(cw=cw, cwf=cwf,
                            inp=np.ascontiguousarray(inp)))
    return in_maps


_NC = None


def kernel(**inputs):
    global _NC
    if _NC is None:
        _NC = build_nc()
    in_maps = shard_inputs(inputs)
    res = bass_utils.run_bass_kernel_spmd(_NC, in_maps,
                                          core_ids=list(range(N_CORES)))
    out = np.stack([res.results[i]["out"] for i in range(M)], axis=0)
    return out.astype(np.float32)
